# revision 9
# baseline (speedup 1.0000x reference)
"""Trainium2 Bass kernel for nn_CrossAttentionModule_bias.

Math (B=2, C=256, H=W=64, N=4096):
    q = queries.reshape(B,C,N).T + q_pos        # [B,N,C]
    k = keys.reshape(B,C,N).T + k_pos
    v = values.reshape(B,C,N).T
    attn = softmax(q @ k.T / sqrt(C)) + c_b     # c_b: per-batch SCALAR
    out  = attn @ v   -> [B,C,H,W]

where c_b = softplus(bias_eye * s_eye) + softplus(bias_mouth * s_mouth) and
s_x = sum(m*m) over the nearest-resized mask (a per-batch scalar).  Adding the
scalar c_b to every attn entry adds c_b * colsum(V) to every output row, i.e.
a rank-1 correction:

    out[n,:] = (sum_m exp(d[n,m]) * v[m,:]) / Z_n + c_b * S       (S = colsum V)

Device kernel (per core, 8 cores = 2 batches x 4 query-column shards):
    dotsT[m,n] = sum_c keff[c,m] * qeff[c,n]     (natural [C,N] layouts)
    e = exp(dotsT * 1/16)                        (no max subtraction; |dots|<~12)
    U_T[c,n] (+ Z[n] via ones-column of vaug) accumulated over m-chunks
    out[c,n] = U_T[c,n] * (1/Z[n]) + cbS[c]

Matmuls run in float32r (full-rate fp32 mode); the 1/Z partition-broadcast
uses an exact plain-fp32 K=1 matmul.
"""

import numpy as np

import concourse.bass as bass
import concourse.mybir as mybir
import concourse.tile as tile
from concourse import bacc
from concourse.bass_utils import run_bass_kernel_spmd

# Problem shape (hardcoded per the task contract)
B, C, H, W = 2, 256, 64, 64
N = H * W                      # 4096
NCORES = 8
SHARDS_PER_B = NCORES // B     # 4 query-column shards per batch
NSH = N // SHARDS_PER_B        # 1024 query columns per core
SCALE = float(C) ** -0.5       # 1/16
P = 128
CCN = C // P                   # 2 c-chunks
MCN = N // P                   # 32 m-chunks
NT_SIZE = 512                  # n-tile width (fp32 moving-operand limit)
NTN = NSH // NT_SIZE           # 2 n-tiles per core

F32 = mybir.dt.float32
F32R = mybir.dt.float32r

EXP = mybir.ActivationFunctionType.Exp

_CACHE: dict = {}


def _build_bass(reps: int = 1, loop_reps: int = 0):
    """reps>1 unrolls the whole compute; loop_reps>0 wraps it in a hardware
    For_i loop instead (timing-only variants: slope between two loop_reps
    builds isolates per-iteration HW time from the ~100ms dispatch floor)."""
    nc = bacc.Bacc("TRN2", target_bir_lowering=False, debug=False)

    keff = nc.dram_tensor("keff", [C, N], F32R, kind="ExternalInput")
    qeff = nc.dram_tensor("qeff", [C, NSH], F32R, kind="ExternalInput")
    vaug = nc.dram_tensor("vaug", [N, C + 1], F32R, kind="ExternalInput")
    cbs = nc.dram_tensor("cbs", [C, 1], F32, kind="ExternalInput")
    out = nc.dram_tensor("out", [C, NSH], F32, kind="ExternalOutput")

    KQ = 8                     # m-chunks per keff DMA tile
    KQN = MCN // KQ            # 4 keff tiles per c-chunk

    with tile.TileContext(nc) as tc:
        with (
            tc.tile_pool(name="const", bufs=1) as cpool,
            tc.tile_pool(name="work", bufs=4) as wpool,
            tc.tile_pool(name="tail", bufs=2) as tpool,
            tc.tile_pool(name="dots_ps", bufs=2, space="PSUM") as dots_pool,
            tc.tile_pool(name="acc_ps", bufs=1, space="PSUM") as acc_pool,
            tc.tile_pool(name="bc_ps", bufs=1, space="PSUM") as bc_pool,
        ):
            zero = cpool.tile([P, 1], F32, tag="zero", name="zero")
            nc.vector.memset(zero[:], 0.0)
            ones_row = cpool.tile([1, P], F32, tag="ones_row", name="ones_row")
            nc.vector.memset(ones_row[:], 1.0)

            cbs_t = []
            for cc in range(CCN):
                t = cpool.tile([P, 1], F32, tag=f"cbs{cc}", name=f"cbs{cc}")
                nc.sync.dma_start(t[:], cbs[cc * P : (cc + 1) * P, :])
                cbs_t.append(t)

            qeff_t = []
            for cc in range(CCN):
                t = cpool.tile([P, NSH], F32R, tag=f"qeff{cc}", name=f"qeff{cc}")
                nc.sync.dma_start(t[:], qeff[cc * P : (cc + 1) * P, :])
                qeff_t.append(t)

            # keff split into [128, KQ*128] tiles so QK can start early
            keff_t = [[None] * KQN for _ in range(CCN)]
            for q in range(KQN):
                for cc in range(CCN):
                    t = cpool.tile([P, KQ * P], F32R, tag=f"keff{cc}_{q}", name=f"keff{cc}_{q}")
                    nc.sync.dma_start(
                        t[:], keff[cc * P : (cc + 1) * P, q * KQ * P : (q + 1) * KQ * P]
                    )
                    keff_t[cc][q] = t

            vaug_t = []
            for mc in range(MCN):
                t = cpool.tile([P, C + 1], F32R, tag=f"vaug{mc}", name=f"vaug{mc}")
                nc.sync.dma_start(t[:], vaug[mc * P : (mc + 1) * P, :])
                vaug_t.append(t)

            def emit_ntile(nt):
                ns = slice(nt * NT_SIZE, (nt + 1) * NT_SIZE)
                u_ps = [
                    acc_pool.tile([P, NT_SIZE], F32, tag=f"u{cc}", name=f"u{cc}")
                    for cc in range(CCN)
                ]
                z_ps = acc_pool.tile([1, NT_SIZE], F32, tag="z", name="z")

                for mc in range(MCN):
                    dots = dots_pool.tile([P, NT_SIZE], F32, tag="dots", name="dots")
                    for cc in range(CCN):
                        lhsT = keff_t[cc][mc // KQ][
                            :, (mc % KQ) * P : (mc % KQ + 1) * P
                        ]
                        nc.tensor.matmul(
                            dots[:],
                            lhsT,
                            qeff_t[cc][:, ns],
                            start=(cc == 0),
                            stop=(cc == CCN - 1),
                        )
                    expt = wpool.tile([P, NT_SIZE], F32R, tag="expt", name="expt")
                    nc.scalar.activation(expt[:], dots[:], EXP, bias=zero[:], scale=SCALE)

                    first, last = mc == 0, mc == MCN - 1
                    for cc in range(CCN):
                        nc.tensor.matmul(
                            u_ps[cc][:],
                            vaug_t[mc][:, cc * P : (cc + 1) * P],
                            expt[:],
                            start=first,
                            stop=last,
                        )
                    nc.tensor.matmul(
                        z_ps[:],
                        vaug_t[mc][:, C : C + 1],
                        expt[:],
                        start=first,
                        stop=last,
                    )

                # normalization: recip + exact fp32 partition-broadcast
                recip = tpool.tile([1, NT_SIZE], F32, tag="recip", name="recip")
                nc.vector.reciprocal(recip[:], z_ps[:])
                bc_ps = bc_pool.tile([P, NT_SIZE], F32, tag="bc", name="bc")
                nc.tensor.matmul(bc_ps[:], ones_row[:], recip[:], start=True, stop=True)
                bc_sb = tpool.tile([P, NT_SIZE], F32, tag="bc_sb", name="bc_sb")
                nc.vector.tensor_copy(bc_sb[:], bc_ps[:])

                for cc in range(CCN):
                    prod = tpool.tile([P, NT_SIZE], F32, tag="prod", name="prod")
                    nc.vector.tensor_mul(prod[:], u_ps[cc][:], bc_sb[:])
                    outsb = tpool.tile([P, NT_SIZE], F32, tag="outsb", name="outsb")
                    nc.vector.tensor_scalar_add(outsb[:], prod[:], cbs_t[cc][:])
                    nc.sync.dma_start(out[cc * P : (cc + 1) * P, ns], outsb[:])

            if loop_reps > 0:
                with tc.For_i(0, loop_reps, 1, hint_engines=(mybir.EngineType.PE,)):
                    for nt in range(NTN):
                        emit_ntile(nt)
            else:
                for _ in range(reps):
                    for nt in range(NTN):
                        emit_ntile(nt)

    nc.compile()
    return nc


def _prep_inputs(queries, keys, values, mask_eye, mask_mouth, q_pos, k_pos,
                 bias_eye, bias_mouth):
    """Host-side shard prep: positional adds, V transpose + ones column, and
    the per-batch scalar bias folded into cbS = c_b * colsum(V)."""
    q = queries.reshape(B, C, N) + q_pos[0].T[None]
    k = keys.reshape(B, C, N) + k_pos[0].T[None]
    vT = np.ascontiguousarray(values.reshape(B, C, N).transpose(0, 2, 1))  # [B,N,C]

    vaug = np.empty((B, N, C + 1), np.float32)
    vaug[:, :, :C] = vT
    vaug[:, :, C] = 1.0

    def msum(mask):
        # nearest resize 128->64 picks every other row/col
        m = mask[:, :, ::2, ::2].reshape(B, -1)
        return (m * m).sum(axis=1, dtype=np.float64)

    softplus = lambda x: np.logaddexp(0.0, x)
    c_b = softplus(float(bias_eye[0]) * msum(mask_eye)) + softplus(
        float(bias_mouth[0]) * msum(mask_mouth)
    )  # [B]
    S = vT.sum(axis=1, dtype=np.float64)  # [B, C]
    cbs = (c_b[:, None] * S).astype(np.float32)  # [B, C]

    in_maps = []
    for core in range(NCORES):
        b, sh = divmod(core, SHARDS_PER_B)
        n0 = sh * NSH
        in_maps.append(
            {
                "keff": np.ascontiguousarray(k[b], np.float32),
                "qeff": np.ascontiguousarray(q[b][:, n0 : n0 + NSH], np.float32),
                "vaug": vaug[b],
                "cbs": np.ascontiguousarray(cbs[b][:, None], np.float32),
            }
        )
    return in_maps


def kernel(**inputs) -> np.ndarray:
    inputs = {k: np.asarray(v, np.float32) for k, v in inputs.items()}
    in_maps = _prep_inputs(**inputs)

    if "nc" not in _CACHE:
        _CACHE["nc"] = _build_bass()
    res = run_bass_kernel_spmd(_CACHE["nc"], in_maps, list(range(NCORES)))

    full = np.empty((B, C, N), np.float32)
    for core in range(NCORES):
        b, sh = divmod(core, SHARDS_PER_B)
        n0 = sh * NSH
        full[b][:, n0 : n0 + NSH] = res.results[core]["out"]
    return full.reshape(B, C, H, W)


# revision 10
# speedup vs baseline: 1.4284x; 1.4284x over previous
"""Trainium2 Bass kernel for nn_CrossAttentionModule_bias.

Math (B=2, C=256, H=W=64, N=4096):
    q = queries.reshape(B,C,N).T + q_pos        # [B,N,C]
    k = keys.reshape(B,C,N).T + k_pos
    v = values.reshape(B,C,N).T
    attn = softmax(q @ k.T / sqrt(C)) + c_b     # c_b: per-batch SCALAR
    out  = attn @ v   -> [B,C,H,W]

where c_b = softplus(bias_eye * s_eye) + softplus(bias_mouth * s_mouth) and
s_x = sum(m*m) over the nearest-resized mask (a per-batch scalar).  Adding the
scalar c_b to every attn entry adds c_b * colsum(V) to every output row, i.e.
a rank-1 correction:

    out[n,:] = (sum_m exp(d[n,m]) * v[m,:]) / Z_n + c_b * S       (S = colsum V)

Device kernel (per core, 8 cores = 2 batches x 4 query-column shards):
    dotsT[m,n] = sum_c keff[c,m] * qeff[c,n]     (natural [C,N] layouts)
    e = exp(dotsT * 1/16)                        (no max subtraction; |dots|<~12)
    U_T[c,n] (+ Z[n] via ones-column of vaug) accumulated over m-chunks
    out[c,n] = U_T[c,n] * (1/Z[n]) + cbS[c]

Matmuls run in float32r (full-rate fp32 mode); the 1/Z partition-broadcast
uses an exact plain-fp32 K=1 matmul.
"""

import numpy as np

import concourse.bass as bass
import concourse.mybir as mybir
import concourse.tile as tile
from concourse import bacc
from concourse.bass_utils import run_bass_kernel_spmd

# Problem shape (hardcoded per the task contract)
B, C, H, W = 2, 256, 64, 64
N = H * W                      # 4096
NCORES = 8
SHARDS_PER_B = NCORES // B     # 4 query-column shards per batch
NSH = N // SHARDS_PER_B        # 1024 query columns per core
SCALE = float(C) ** -0.5       # 1/16
P = 128
CCN = C // P                   # 2 c-chunks
MCN = N // P                   # 32 m-chunks
NT_SIZE = 512                  # n-tile width (fp32 moving-operand limit)
NTN = NSH // NT_SIZE           # 2 n-tiles per core

F32 = mybir.dt.float32
F32R = mybir.dt.float32r

EXP = mybir.ActivationFunctionType.Exp

_CACHE: dict = {}


def _build_bass(reps: int = 1, loop_reps: int = 0):
    """reps>1 unrolls the whole compute; loop_reps>0 wraps it in a hardware
    For_i loop instead (timing-only variants: slope between two loop_reps
    builds isolates per-iteration HW time from the ~100ms dispatch floor)."""
    nc = bacc.Bacc("TRN2", target_bir_lowering=False, debug=False)

    keff = nc.dram_tensor("keff", [C, N], F32R, kind="ExternalInput")
    qeff = nc.dram_tensor("qeff", [C, NSH], F32R, kind="ExternalInput")
    vaug = nc.dram_tensor("vaug", [N, C + 1], F32R, kind="ExternalInput")
    cbs = nc.dram_tensor("cbs", [C, 1], F32, kind="ExternalInput")
    out = nc.dram_tensor("out", [C, NSH], F32, kind="ExternalOutput")

    KQ = 8                     # m-chunks per keff DMA tile
    KQN = MCN // KQ            # 4 keff tiles per c-chunk

    with tile.TileContext(nc) as tc:
        with (
            tc.tile_pool(name="const", bufs=1) as cpool,
            tc.tile_pool(name="work", bufs=4) as wpool,
            tc.tile_pool(name="tail", bufs=2) as tpool,
            tc.tile_pool(name="dots_ps", bufs=2, space="PSUM") as dots_pool,
            tc.tile_pool(name="acc_ps", bufs=1, space="PSUM") as acc_pool,
            tc.tile_pool(name="bc_ps", bufs=1, space="PSUM") as bc_pool,
        ):
            zero = cpool.tile([P, 1], F32, tag="zero", name="zero")
            nc.vector.memset(zero[:], 0.0)
            ones_row = cpool.tile([1, P], F32, tag="ones_row", name="ones_row")
            nc.vector.memset(ones_row[:], 1.0)

            cbs_t = []
            for cc in range(CCN):
                t = cpool.tile([P, 1], F32, tag=f"cbs{cc}", name=f"cbs{cc}")
                nc.sync.dma_start(t[:], cbs[cc * P : (cc + 1) * P, :])
                cbs_t.append(t)

            qeff_t = []
            for cc in range(CCN):
                t = cpool.tile([P, NSH], F32R, tag=f"qeff{cc}", name=f"qeff{cc}")
                nc.sync.dma_start(t[:], qeff[cc * P : (cc + 1) * P, :])
                qeff_t.append(t)

            # keff split into [128, KQ*128] tiles so QK can start early
            keff_t = [[None] * KQN for _ in range(CCN)]
            for q in range(KQN):
                for cc in range(CCN):
                    t = cpool.tile([P, KQ * P], F32R, tag=f"keff{cc}_{q}", name=f"keff{cc}_{q}")
                    nc.sync.dma_start(
                        t[:], keff[cc * P : (cc + 1) * P, q * KQ * P : (q + 1) * KQ * P]
                    )
                    keff_t[cc][q] = t

            vaug_t = []
            for mc in range(MCN):
                t = cpool.tile([P, C + 1], F32R, tag=f"vaug{mc}", name=f"vaug{mc}")
                nc.sync.dma_start(t[:], vaug[mc * P : (mc + 1) * P, :])
                vaug_t.append(t)

            def emit_qk(nt, mc):
                ns = slice(nt * NT_SIZE, (nt + 1) * NT_SIZE)
                dots = dots_pool.tile([P, NT_SIZE], F32, tag="dots", name="dots")
                for cc in range(CCN):
                    lhsT = keff_t[cc][mc // KQ][:, (mc % KQ) * P : (mc % KQ + 1) * P]
                    nc.tensor.matmul(
                        dots[:],
                        lhsT,
                        qeff_t[cc][:, ns],
                        start=(cc == 0),
                        stop=(cc == CCN - 1),
                    )
                return dots

            def emit_body():
                # u/z accumulators for both n-tiles live across the whole
                # m-loop: dots(2) + u(4) + z(2) = 8 PSUM banks
                u_ps = [
                    [
                        acc_pool.tile([P, NT_SIZE], F32, tag=f"u{cc}n{nt}", name=f"u{cc}n{nt}")
                        for cc in range(CCN)
                    ]
                    for nt in range(NTN)
                ]
                z_ps = [
                    acc_pool.tile([1, NT_SIZE], F32, tag=f"zn{nt}", name=f"zn{nt}")
                    for nt in range(NTN)
                ]

                # software-pipelined: PE gets QK(i+1) between exp(i) and AV(i),
                # so it never idles while ACT computes exp
                steps = [(nt, mc) for nt in range(NTN) for mc in range(MCN)]
                dots = emit_qk(*steps[0])
                for i, (nt, mc) in enumerate(steps):
                    expt = wpool.tile([P, NT_SIZE], F32R, tag="expt", name="expt")
                    nc.scalar.activation(expt[:], dots[:], EXP, bias=zero[:], scale=SCALE)
                    if i + 1 < len(steps):
                        dots = emit_qk(*steps[i + 1])
                    first, last = mc == 0, mc == MCN - 1
                    for cc in range(CCN):
                        nc.tensor.matmul(
                            u_ps[nt][cc][:],
                            vaug_t[mc][:, cc * P : (cc + 1) * P],
                            expt[:],
                            start=first,
                            stop=last,
                        )
                    nc.tensor.matmul(
                        z_ps[nt][:],
                        vaug_t[mc][:, C : C + 1],
                        expt[:],
                        start=first,
                        stop=last,
                    )

                # normalization: recip + exact fp32 partition-broadcast
                for nt in range(NTN):
                    ns = slice(nt * NT_SIZE, (nt + 1) * NT_SIZE)
                    recip = tpool.tile([1, NT_SIZE], F32, tag="recip", name="recip")
                    nc.vector.reciprocal(recip[:], z_ps[nt][:])
                    bc_ps = dots_pool.tile([P, NT_SIZE], F32, tag="dots", name="bc")
                    nc.tensor.matmul(bc_ps[:], ones_row[:], recip[:], start=True, stop=True)
                    bc_sb = tpool.tile([P, NT_SIZE], F32, tag="bc_sb", name="bc_sb")
                    nc.vector.tensor_copy(bc_sb[:], bc_ps[:])

                    for cc in range(CCN):
                        prod = tpool.tile([P, NT_SIZE], F32, tag="prod", name="prod")
                        nc.vector.tensor_mul(prod[:], u_ps[nt][cc][:], bc_sb[:])
                        outsb = tpool.tile([P, NT_SIZE], F32, tag="outsb", name="outsb")
                        nc.vector.tensor_scalar_add(outsb[:], prod[:], cbs_t[cc][:])
                        nc.sync.dma_start(out[cc * P : (cc + 1) * P, ns], outsb[:])

            if loop_reps > 0:
                with tc.For_i(0, loop_reps, 1, hint_engines=(mybir.EngineType.PE,)):
                    emit_body()
            else:
                for _ in range(reps):
                    emit_body()

    nc.compile()
    return nc


def _prep_inputs(queries, keys, values, mask_eye, mask_mouth, q_pos, k_pos,
                 bias_eye, bias_mouth):
    """Host-side shard prep: positional adds, V transpose + ones column, and
    the per-batch scalar bias folded into cbS = c_b * colsum(V)."""
    q = queries.reshape(B, C, N) + q_pos[0].T[None]
    k = keys.reshape(B, C, N) + k_pos[0].T[None]
    vT = np.ascontiguousarray(values.reshape(B, C, N).transpose(0, 2, 1))  # [B,N,C]

    vaug = np.empty((B, N, C + 1), np.float32)
    vaug[:, :, :C] = vT
    vaug[:, :, C] = 1.0

    def msum(mask):
        # nearest resize 128->64 picks every other row/col
        m = mask[:, :, ::2, ::2].reshape(B, -1)
        return (m * m).sum(axis=1, dtype=np.float64)

    softplus = lambda x: np.logaddexp(0.0, x)
    c_b = softplus(float(bias_eye[0]) * msum(mask_eye)) + softplus(
        float(bias_mouth[0]) * msum(mask_mouth)
    )  # [B]
    S = vT.sum(axis=1, dtype=np.float64)  # [B, C]
    cbs = (c_b[:, None] * S).astype(np.float32)  # [B, C]

    in_maps = []
    for core in range(NCORES):
        b, sh = divmod(core, SHARDS_PER_B)
        n0 = sh * NSH
        in_maps.append(
            {
                "keff": np.ascontiguousarray(k[b], np.float32),
                "qeff": np.ascontiguousarray(q[b][:, n0 : n0 + NSH], np.float32),
                "vaug": vaug[b],
                "cbs": np.ascontiguousarray(cbs[b][:, None], np.float32),
            }
        )
    return in_maps


def kernel(**inputs) -> np.ndarray:
    inputs = {k: np.asarray(v, np.float32) for k, v in inputs.items()}
    in_maps = _prep_inputs(**inputs)

    if "nc" not in _CACHE:
        _CACHE["nc"] = _build_bass()
    res = run_bass_kernel_spmd(_CACHE["nc"], in_maps, list(range(NCORES)))

    full = np.empty((B, C, N), np.float32)
    for core in range(NCORES):
        b, sh = divmod(core, SHARDS_PER_B)
        n0 = sh * NSH
        full[b][:, n0 : n0 + NSH] = res.results[core]["out"]
    return full.reshape(B, C, H, W)


# revision 14
# speedup vs baseline: 1.4462x; 1.0124x over previous
"""Trainium2 Bass kernel for nn_CrossAttentionModule_bias.

Math (B=2, C=256, H=W=64, N=4096):
    q = queries.reshape(B,C,N).T + q_pos        # [B,N,C]
    k = keys.reshape(B,C,N).T + k_pos
    v = values.reshape(B,C,N).T
    attn = softmax(q @ k.T / sqrt(C)) + c_b     # c_b: per-batch SCALAR
    out  = attn @ v   -> [B,C,H,W]

where c_b = softplus(bias_eye * s_eye) + softplus(bias_mouth * s_mouth) and
s_x = sum(m*m) over the nearest-resized mask (a per-batch scalar).  Adding the
scalar c_b to every attn entry adds c_b * colsum(V) to every output row, i.e.
a rank-1 correction:

    out[n,:] = (sum_m exp(d[n,m]) * v[m,:]) / Z_n + c_b * S       (S = colsum V)

Device kernel (per core, 8 cores = 2 batches x 4 query-column shards):
    dotsT[m,n] = sum_c keff[c,m] * qeff[c,n]     (natural [C,N] layouts)
    e = exp(dotsT * 1/16)                        (no max subtraction; |dots|<~12)
    U_T[c,n] (+ Z[n] via ones-column of vaug) accumulated over m-chunks
    out[c,n] = U_T[c,n] * (1/Z[n]) + cbS[c]

Matmuls run in float32r (full-rate fp32 mode); the 1/Z partition-broadcast
uses an exact plain-fp32 K=1 matmul.
"""

import numpy as np

import concourse.bass as bass
import concourse.mybir as mybir
import concourse.tile as tile
from concourse import bacc
from concourse.bass_utils import run_bass_kernel_spmd

# Problem shape (hardcoded per the task contract)
B, C, H, W = 2, 256, 64, 64
N = H * W                      # 4096
NCORES = 8
SHARDS_PER_B = NCORES // B     # 4 query-column shards per batch
NSH = N // SHARDS_PER_B        # 1024 query columns per core
SCALE = float(C) ** -0.5       # 1/16
P = 128
CCN = C // P                   # 2 c-chunks
MCN = N // P                   # 32 m-chunks
NT_SIZE = 512                  # n-tile width (fp32 moving-operand limit)
NTN = NSH // NT_SIZE           # 2 n-tiles per core

F32 = mybir.dt.float32
F32R = mybir.dt.float32r

EXP = mybir.ActivationFunctionType.Exp

_CACHE: dict = {}


def _build_bass(reps: int = 1, loop_reps: int = 0, ablate: tuple = (), seqnt: bool = False, z1bank: bool = False):
    """reps>1 unrolls the whole compute; loop_reps>0 wraps it in a hardware
    For_i loop instead (timing-only variants: slope between two loop_reps
    builds isolates per-iteration HW time from the ~100ms dispatch floor)."""
    nc = bacc.Bacc("TRN2", target_bir_lowering=False, debug=False)

    keff = nc.dram_tensor("keff", [C, N], F32R, kind="ExternalInput")
    qeff = nc.dram_tensor("qeff", [C, NSH], F32R, kind="ExternalInput")
    vaug = nc.dram_tensor("vaug", [N, C + 1], F32R, kind="ExternalInput")
    cbs = nc.dram_tensor("cbs", [C, 1], F32, kind="ExternalInput")
    out = nc.dram_tensor("out", [C, NSH], F32, kind="ExternalOutput")

    KQ = 8                     # m-chunks per keff DMA tile
    KQN = MCN // KQ            # 4 keff tiles per c-chunk

    with tile.TileContext(nc) as tc:
        with (
            tc.tile_pool(name="const", bufs=1) as cpool,
            tc.tile_pool(name="work", bufs=4) as wpool,
            tc.tile_pool(name="tail", bufs=2) as tpool,
            tc.tile_pool(name="dots_ps", bufs=(4 if seqnt else (3 if z1bank else 2)), space="PSUM") as dots_pool,
            tc.tile_pool(name="acc_ps", bufs=1, space="PSUM") as acc_pool,
            tc.tile_pool(name="bc_ps", bufs=1, space="PSUM") as bc_pool,
        ):
            zero = cpool.tile([P, 1], F32, tag="zero", name="zero")
            nc.vector.memset(zero[:], 0.0)
            ones_row = cpool.tile([1, P], F32, tag="ones_row", name="ones_row")
            nc.vector.memset(ones_row[:], 1.0)

            cbs_t = []
            for cc in range(CCN):
                t = cpool.tile([P, 1], F32, tag=f"cbs{cc}", name=f"cbs{cc}")
                nc.sync.dma_start(t[:], cbs[cc * P : (cc + 1) * P, :])
                cbs_t.append(t)

            qeff_t = []
            for cc in range(CCN):
                t = cpool.tile([P, NSH], F32R, tag=f"qeff{cc}", name=f"qeff{cc}")
                nc.sync.dma_start(t[:], qeff[cc * P : (cc + 1) * P, :])
                qeff_t.append(t)

            # keff split into [128, KQ*128] tiles so QK can start early
            keff_t = [[None] * KQN for _ in range(CCN)]
            for q in range(KQN):
                for cc in range(CCN):
                    t = cpool.tile([P, KQ * P], F32R, tag=f"keff{cc}_{q}", name=f"keff{cc}_{q}")
                    nc.sync.dma_start(
                        t[:], keff[cc * P : (cc + 1) * P, q * KQ * P : (q + 1) * KQ * P]
                    )
                    keff_t[cc][q] = t

            vaug_t = []
            for mc in range(MCN):
                t = cpool.tile([P, C + 1], F32R, tag=f"vaug{mc}", name=f"vaug{mc}")
                nc.sync.dma_start(t[:], vaug[mc * P : (mc + 1) * P, :])
                vaug_t.append(t)

            const_expt = None
            if "exp" in ablate:
                const_expt = cpool.tile([P, NT_SIZE], F32R, tag="cexpt", name="cexpt")
                nc.vector.memset(const_expt[:], 1.0)

            def emit_qk(nt, mc):
                ns = slice(nt * NT_SIZE, (nt + 1) * NT_SIZE)
                dots = dots_pool.tile([P, NT_SIZE], F32, tag="dots", name="dots")
                for cc in range(CCN):
                    lhsT = keff_t[cc][mc // KQ][:, (mc % KQ) * P : (mc % KQ + 1) * P]
                    nc.tensor.matmul(
                        dots[:],
                        lhsT,
                        qeff_t[cc][:, ns],
                        start=(cc == 0),
                        stop=(cc == CCN - 1),
                    )
                return dots

            def emit_body():
                # u/z accumulators for both n-tiles live across the whole
                # m-loop: dots(2) + u(4) + z(2) = 8 PSUM banks
                zshared = (
                    acc_pool.tile([P, NT_SIZE], F32, tag="zsh", name="zsh")
                    if z1bank
                    else None
                )

                def alloc_acc(nt):
                    sfx = "" if seqnt else f"n{nt}"
                    u = [
                        acc_pool.tile([P, NT_SIZE], F32, tag=f"u{cc}{sfx}", name=f"u{cc}{sfx}")
                        for cc in range(CCN)
                    ]
                    if z1bank:
                        z = zshared[nt * 32 : nt * 32 + 1, :]
                    else:
                        z = acc_pool.tile([1, NT_SIZE], F32, tag=f"z{sfx}", name=f"z{sfx}")
                    return u, z

                if seqnt:
                    u_ps, z_ps = [None] * NTN, [None] * NTN
                    u_ps[0], z_ps[0] = alloc_acc(0)
                else:
                    accs = [alloc_acc(nt) for nt in range(NTN)]
                    u_ps = [a[0] for a in accs]
                    z_ps = [a[1] for a in accs]

                def emit_tail(nt):
                    # normalization: recip + exact fp32 partition-broadcast,
                    # overlapped with the next n-tile's m-loop
                    ns = slice(nt * NT_SIZE, (nt + 1) * NT_SIZE)
                    recip = tpool.tile([1, NT_SIZE], F32, tag="recip", name="recip")
                    nc.vector.reciprocal(recip[:], z_ps[nt][:])
                    if seqnt:
                        bc_ps = bc_pool.tile([P, NT_SIZE], F32, tag="bc", name="bc")
                    else:
                        bc_ps = dots_pool.tile([P, NT_SIZE], F32, tag="dots", name="bc")
                    nc.tensor.matmul(bc_ps[:], ones_row[:], recip[:], start=True, stop=True)
                    bc_sb = tpool.tile([P, NT_SIZE], F32, tag="bc_sb", name="bc_sb")
                    nc.vector.tensor_copy(bc_sb[:], bc_ps[:])

                    for cc in range(CCN):
                        prod = tpool.tile([P, NT_SIZE], F32, tag="prod", name="prod")
                        nc.vector.tensor_mul(prod[:], u_ps[nt][cc][:], bc_sb[:])
                        outsb = tpool.tile([P, NT_SIZE], F32, tag="outsb", name="outsb")
                        nc.vector.tensor_scalar_add(outsb[:], prod[:], cbs_t[cc][:])
                        nc.sync.dma_start(out[cc * P : (cc + 1) * P, ns], outsb[:])

                # software-pipelined: PE gets QK(i+1) between exp(i) and AV(i),
                # so it never idles while ACT computes exp
                steps = [(nt, mc) for nt in range(NTN) for mc in range(MCN)]
                dots = emit_qk(*steps[0])
                for i, (nt, mc) in enumerate(steps):
                    if "exp" in ablate:
                        expt = const_expt
                    else:
                        expt = wpool.tile([P, NT_SIZE], F32R, tag="expt", name="expt")
                        nc.scalar.activation(expt[:], dots[:], EXP, bias=zero[:], scale=SCALE)
                    if i + 1 < len(steps):
                        dots = emit_qk(*steps[i + 1])
                    first, last = mc == 0, mc == MCN - 1
                    for cc in range(CCN):
                        nc.tensor.matmul(
                            u_ps[nt][cc][:],
                            vaug_t[mc][:, cc * P : (cc + 1) * P],
                            expt[:],
                            start=first,
                            stop=last,
                        )
                    if "z" not in ablate:
                        nc.tensor.matmul(
                            z_ps[nt][:],
                            vaug_t[mc][:, C : C + 1],
                            expt[:],
                            start=first,
                            stop=last,
                            tile_position=((0, nt * 32) if z1bank else None),
                        )
                    if last and seqnt and "tail" not in ablate:
                        emit_tail(nt)
                    if last and seqnt and nt + 1 < NTN:
                        u_ps[nt + 1], z_ps[nt + 1] = alloc_acc(nt + 1)



                if not seqnt and "tail" not in ablate:
                    for nt in range(NTN):
                        emit_tail(nt)

            if loop_reps > 0:
                with tc.For_i(0, loop_reps, 1, hint_engines=(mybir.EngineType.PE,)):
                    emit_body()
            else:
                for _ in range(reps):
                    emit_body()

    nc.compile()
    return nc


def _prep_inputs(queries, keys, values, mask_eye, mask_mouth, q_pos, k_pos,
                 bias_eye, bias_mouth):
    """Host-side shard prep: positional adds, V transpose + ones column, and
    the per-batch scalar bias folded into cbS = c_b * colsum(V)."""
    q = queries.reshape(B, C, N) + q_pos[0].T[None]
    k = keys.reshape(B, C, N) + k_pos[0].T[None]
    vT = np.ascontiguousarray(values.reshape(B, C, N).transpose(0, 2, 1))  # [B,N,C]

    vaug = np.empty((B, N, C + 1), np.float32)
    vaug[:, :, :C] = vT
    vaug[:, :, C] = 1.0

    def msum(mask):
        # nearest resize 128->64 picks every other row/col
        m = mask[:, :, ::2, ::2].reshape(B, -1)
        return (m * m).sum(axis=1, dtype=np.float64)

    softplus = lambda x: np.logaddexp(0.0, x)
    c_b = softplus(float(bias_eye[0]) * msum(mask_eye)) + softplus(
        float(bias_mouth[0]) * msum(mask_mouth)
    )  # [B]
    S = vT.sum(axis=1, dtype=np.float64)  # [B, C]
    cbs = (c_b[:, None] * S).astype(np.float32)  # [B, C]

    in_maps = []
    for core in range(NCORES):
        b, sh = divmod(core, SHARDS_PER_B)
        n0 = sh * NSH
        in_maps.append(
            {
                "keff": np.ascontiguousarray(k[b], np.float32),
                "qeff": np.ascontiguousarray(q[b][:, n0 : n0 + NSH], np.float32),
                "vaug": vaug[b],
                "cbs": np.ascontiguousarray(cbs[b][:, None], np.float32),
            }
        )
    return in_maps


def kernel(**inputs) -> np.ndarray:
    inputs = {k: np.asarray(v, np.float32) for k, v in inputs.items()}
    in_maps = _prep_inputs(**inputs)

    if "nc" not in _CACHE:
        _CACHE["nc"] = _build_bass()
    res = run_bass_kernel_spmd(_CACHE["nc"], in_maps, list(range(NCORES)))

    full = np.empty((B, C, N), np.float32)
    for core in range(NCORES):
        b, sh = divmod(core, SHARDS_PER_B)
        n0 = sh * NSH
        full[b][:, n0 : n0 + NSH] = res.results[core]["out"]
    return full.reshape(B, C, H, W)
